# revision 27
# baseline (speedup 1.0000x reference)
"""Trainium2 Bass kernel for attention-MIL segment-reduce (raw Bass, SPMD x8).

Math: with e_i = exp(logits_i) (|logits| <= 5.9 so no max-subtraction
needed; the +ba2 shift cancels in the softmax), per-core partials
    Xs[s, :] = sum_{i in s} e_i * x_i      es[s] = sum_{i in s} e_i
give M[s] = (Xs[s]/es[s]) @ W1 + b1 on the host (tiny [64,.] epilogue).
logits_i = tanh(x_i @ Wc + bc) @ Wa2 with Wc = W1@Wa1 fused on host, so
the [N, 256] hidden H is never materialized; x is read exactly once.

Device pipeline per 256-row block (128 blocks/core, explicit semaphores):
  SYNC: DMA x block [128, 2, 1024] (1 MiB contiguous)
  PE  : transpose x -> xT (16x [128,128], fp32r), g^T = Wc^T @ xT,
        logits^T = Wa2^T @ tanh, e-transpose (K=1), Xs/es accumulation
  ACT : xT copies (half), tanh, exp
  DVE : xT copies (half), logits copy, P[r,s] = (iota==seg)*e
All matmuls are fp32r (full-rate reduced-precision fp32; bits = fp32).
Raw Bass is used (not Tile): walrus allows one sync-wait per engine
instruction, which Tile's scheduler cannot guarantee for this pipeline;
here every wait is a standalone wait_ge.
"""

import os
import numpy as np

import concourse.bass as bass
from concourse import mybir
from concourse.bass_utils import run_bass_kernel_spmd

# ---- problem constants (hardcoded per contract) ----
N = 262144
L = 1024
S = 64   # segments
F = 32   # attention hidden
N_CORES = 8
R = N // N_CORES          # rows per core = 32768
TPB = 2                   # row-tiles (128 rows) per block
LC = L // 128             # feature chunks = 8

F32 = mybir.dt.float32
F32R = mybir.dt.float32r
MD = F32R                 # matmul operand dtype


def build_program(n_rows=R, iters=1):
    nb = n_rows // (128 * TPB)
    nc = bass.Bass()

    x_d = nc.declare_dram_parameter("x", [n_rows, L], MD, isOutput=False)
    seg_d = nc.declare_dram_parameter("segf", [128, n_rows // 128], F32, isOutput=False)
    wc_d = nc.declare_dram_parameter("wc", [L, F], MD, isOutput=False)
    bc_d = nc.declare_dram_parameter("bc", [F, 1], F32, isOutput=False)
    wa2_d = nc.declare_dram_parameter("wa2", [F, 1], MD, isOutput=False)
    ident_d = nc.declare_dram_parameter("ident", [128, 128], MD, isOutput=False)
    iota_d = nc.declare_dram_parameter("iota64", [128, S], F32, isOutput=False)
    ones_d = nc.declare_dram_parameter("ones", [128, 2], MD, isOutput=False)
    xs_d = nc.declare_dram_parameter("xs", [S, L], F32, isOutput=True)
    es_d = nc.declare_dram_parameter("es", [S, 1], F32, isOutput=True)

    x_v = x_d[:].rearrange("(nb t p) l -> nb p t l", t=TPB, p=128)

    from contextlib import ExitStack

    with ExitStack() as ctx:
        ident = ctx.enter_context(nc.sbuf_tensor([128, 128], MD))
        wc_sb = ctx.enter_context(nc.sbuf_tensor([128, LC, F], MD))
        bc_sb = ctx.enter_context(nc.sbuf_tensor([F, 1], F32))
        wa2_sb = ctx.enter_context(nc.sbuf_tensor([F, 1], MD))
        iota_sb = ctx.enter_context(nc.sbuf_tensor([128, S], F32))
        ones_sb = ctx.enter_context(nc.sbuf_tensor([128, 2], MD))
        seg_sb = ctx.enter_context(nc.sbuf_tensor([128, n_rows // 128], F32))
        xb = ctx.enter_context(nc.sbuf_tensor([128, 2, TPB, L], MD))
        xt = ctx.enter_context(nc.sbuf_tensor([128, 2, LC, TPB * 128], MD))
        t_sb = ctx.enter_context(nc.sbuf_tensor([F, 2, TPB * 128], MD))
        lg_sb = ctx.enter_context(nc.sbuf_tensor([1, 2, TPB * 128], MD))
        e_sb = ctx.enter_context(nc.sbuf_tensor([128, 2, 2 * TPB], F32))
        p_sb = ctx.enter_context(nc.sbuf_tensor([128, 2, TPB, S], MD))
        xs_sb = ctx.enter_context(nc.sbuf_tensor([S, L], F32))
        es_sb = ctx.enter_context(nc.sbuf_tensor([S, 1], F32))
        bank_xs0 = ctx.enter_context(nc.psum_tensor([128, 512], F32))
        bank_xs1 = ctx.enter_context(nc.psum_tensor([128, 512], F32))
        xtpA = ctx.enter_context(nc.psum_tensor([128, 512], MD))
        xtpB = ctx.enter_context(nc.psum_tensor([128, 512], MD))
        xtpC = ctx.enter_context(nc.psum_tensor([128, 512], MD))
        xtpD = ctx.enter_context(nc.psum_tensor([128, 512], MD))
        bank_attn = ctx.enter_context(nc.psum_tensor([128, 512], F32))
        bank_en = ctx.enter_context(nc.psum_tensor([128, 512], F32))
        s_cdma = ctx.enter_context(nc.semaphore("s_cdma"))
        s_dma0 = ctx.enter_context(nc.semaphore("s_dma0"))
        s_dma1 = ctx.enter_context(nc.semaphore("s_dma1"))
        s_odma = ctx.enter_context(nc.semaphore("s_odma"))
        s_pe = ctx.enter_context(nc.semaphore("s_pe"))
        s_act = ctx.enter_context(nc.semaphore("s_act"))
        s_dve = ctx.enter_context(nc.semaphore("s_dve"))
        block = ctx.enter_context(nc.Block())

        xs_ps0 = bank_xs0[0:S, :]
        xs_ps1 = bank_xs1[0:S, :]
        es_ps = bank_xs0[64 : 64 + S, 0:2]
        g_ps = bank_attn[0:F, 0 : TPB * 128]
        lg_ps = bank_attn[0:1, 256 : 256 + TPB * 128]
        en_ps = bank_en[0:128, 0 : 2 * TPB]
        xtp = [[xtpA, xtpB], [xtpC, xtpD]]  # [t][half]

        # semaphore targets per block i:
        # PE:  t0T=6i+1, t1T=6i+2, g=6i+3, logits=6i+4, en=6i+5, xs=6i+6
        # ACT: t0h0cp=4i+1, t1h0cp=4i+2, tanh=4i+3, exp=4i+4
        # DVE: t0h1cp=5i+1, t1h1cp=5i+2, lgcp=5i+3, P0=5i+4, P1=5i+5
        NBI = nb * iters

        @block.sync
        def _(sync):
            for dst, src in (
                (ident[:], ident_d[:]),
                (wc_sb[:], wc_d[:].rearrange("(c p) f -> p c f", p=128)),
                (bc_sb[:], bc_d[:]),
                (wa2_sb[:], wa2_d[:]),
                (iota_sb[:], iota_d[:]),
                (ones_sb[:], ones_d[:]),
                (seg_sb[:], seg_d[:]),
            ):
                sync.dma_start(out=dst, in_=src).then_inc(s_cdma, 16)
            s_dmas = (s_dma0, s_dma1)
            for rep in range(iters):
                for b in range(nb):
                    i = rep * nb + b
                    if i >= 2:
                        # xb[i%2] free once Xs of block i-2 finished
                        sync.wait_ge(s_pe, 6 * (i - 2) + 6)
                        # flow control: previous DMA on this parity done
                        sync.wait_ge(s_dmas[i % 2], 16 * (i // 2))
                    sync.dma_start(
                        out=xb[:, i % 2], in_=x_v[b]
                    ).then_inc(s_dmas[i % 2], 16)
            # final outputs
            sync.wait_ge(s_dve, 5 * NBI + 1)
            sync.dma_start(out=xs_d[:], in_=xs_sb[:]).then_inc(s_odma, 16)
            sync.wait_ge(s_odma, 16)
            sync.wait_ge(s_act, 4 * NBI + 1)
            sync.dma_start(out=es_d[:], in_=es_sb[:]).then_inc(s_odma, 16)
            sync.wait_ge(s_odma, 32)

        @block.tensor
        def _(tensor):
            tensor.wait_ge(s_cdma, 7 * 16)
            for rep in range(iters):
                for b in range(nb):
                    i = rep * nb + b
                    p = i % 2
                    first = i == 0
                    last = i == NBI - 1
                    tensor.wait_ge((s_dma0, s_dma1)[i % 2], 16 * (i // 2 + 1))
                    for t in range(TPB):
                        for half in range(2):
                            for j in range(4):
                                c = half * 4 + j
                                mm = nc.tensor.transpose(
                                    xtp[t][half][:, 128 * j : 128 * (j + 1)],
                                    xb[:, p, t, 128 * c : 128 * (c + 1)],
                                    ident[:],
                                )
                        # matmuls complete in pc order; inc rides the last
                        mm.then_inc(s_pe, 1)               # 6i+1 / 6i+2
                    # g matmuls need all 4 xt copies of this block
                    tensor.wait_ge(s_act, 4 * i + 2)
                    tensor.wait_ge(s_dve, 5 * i + 2)
                    for c in range(LC):
                        mm = nc.tensor.matmul(
                            g_ps,
                            wc_sb[:, c, :],
                            xt[:, p, c, :],
                            start=(c == 0),
                            stop=(c == LC - 1),
                        )
                    mm.then_inc(s_pe, 1)               # 6i+3: g done
                    tensor.wait_ge(s_act, 4 * i + 3)   # tanh done
                    nc.tensor.matmul(
                        lg_ps, wa2_sb[:], t_sb[:, p, :], start=True, stop=True
                    ).then_inc(s_pe, 1)                # 6i+4: logits done
                    tensor.wait_ge(s_dve, 5 * i + 3)   # lg copy done
                    for t in range(TPB):
                        mm = nc.tensor.matmul(
                            en_ps[:, 2 * t : 2 * t + 2],
                            lg_sb[0:1, p, t * 128 : (t + 1) * 128],
                            ones_sb[0:1, 0:2],
                            start=True,
                            stop=True,
                        )
                        if t == TPB - 1:
                            mm.then_inc(s_pe, 1)       # 6i+5: en done
                    tensor.wait_ge(s_dve, 5 * i + 5)   # both P built
                    for t in range(TPB):
                        st = first and t == 0
                        sp = last and t == TPB - 1
                        nc.tensor.matmul(
                            xs_ps0,
                            p_sb[:, p, t, :],
                            xb[:, p, t, 0:512],
                            start=st, stop=sp, skip_group_check=True,
                        )
                        nc.tensor.matmul(
                            xs_ps1,
                            p_sb[:, p, t, :],
                            xb[:, p, t, 512:1024],
                            start=st, stop=sp, skip_group_check=True,
                        )
                        mm = nc.tensor.matmul(
                            es_ps,
                            p_sb[:, p, t, :].bitcast(F32),
                            ones_sb[:, 0:2].bitcast(F32),
                            start=st, stop=sp, skip_group_check=True,
                        )
                        if t == TPB - 1:
                            mm.then_inc(s_pe, 1)       # 6i+6: xs done

        @block.scalar
        def _(scalar):
            scalar.wait_ge(s_cdma, 7 * 16)
            for rep in range(iters):
                for b in range(nb):
                    i = rep * nb + b
                    p = i % 2
                    scalar.wait_ge(s_pe, 6 * i + 1)
                    nc.scalar.copy(
                        out=xt[:, p, 0:4, 0:128],
                        in_=xtpA[:].rearrange("q (c r) -> q c r", c=4),
                    ).then_inc(s_act, 1)               # 4i+1
                    scalar.wait_ge(s_pe, 6 * i + 2)
                    nc.scalar.copy(
                        out=xt[:, p, 0:4, 128:256],
                        in_=xtpC[:].rearrange("q (c r) -> q c r", c=4),
                    ).then_inc(s_act, 1)               # 4i+2
                    scalar.wait_ge(s_pe, 6 * i + 3)
                    nc.scalar.activation(
                        out=t_sb[:, p, :], in_=g_ps,
                        func=mybir.ActivationFunctionType.Tanh,
                        bias=bc_sb[:], scale=1.0,
                    ).then_inc(s_act, 1)               # 4i+3
                    scalar.wait_ge(s_pe, 6 * i + 5)
                    nc.scalar.activation(
                        out=e_sb[:, p, :], in_=en_ps,
                        func=mybir.ActivationFunctionType.Exp,
                    ).then_inc(s_act, 1)               # 4i+4
            scalar.wait_ge(s_pe, 6 * NBI)
            nc.scalar.copy(out=es_sb[:], in_=es_ps[:, 0:1]).then_inc(s_act, 1)

        @block.vector
        def _(vector):
            vector.wait_ge(s_cdma, 7 * 16)
            for rep in range(iters):
                for b in range(nb):
                    i = rep * nb + b
                    p = i % 2
                    vector.wait_ge(s_pe, 6 * i + 1)
                    nc.vector.tensor_copy(
                        out=xt[:, p, 4:8, 0:128],
                        in_=xtpB[:].rearrange("q (c r) -> q c r", c=4),
                    ).then_inc(s_dve, 1)               # 5i+1
                    vector.wait_ge(s_pe, 6 * i + 2)
                    nc.vector.tensor_copy(
                        out=xt[:, p, 4:8, 128:256],
                        in_=xtpD[:].rearrange("q (c r) -> q c r", c=4),
                    ).then_inc(s_dve, 1)               # 5i+2
                    vector.wait_ge(s_pe, 6 * i + 4)
                    nc.vector.tensor_copy(
                        out=lg_sb[:, p, :], in_=lg_ps
                    ).then_inc(s_dve, 1)               # 5i+3
                    vector.wait_ge(s_act, 4 * i + 4)   # exp done
                    if i >= 2:
                        vector.wait_ge(s_pe, 6 * (i - 2) + 6)  # p_sb free
                    for t in range(TPB):
                        nc.vector.tensor_scalar(
                            out=p_sb[:, p, t, :],
                            in0=iota_sb[:],
                            scalar1=seg_sb[:, b * TPB + t : b * TPB + t + 1],
                            scalar2=e_sb[:, p, 2 * t : 2 * t + 1],
                            op0=mybir.AluOpType.is_equal,
                            op1=mybir.AluOpType.mult,
                        ).then_inc(s_dve, 1)           # 5i+4, 5i+5
            vector.wait_ge(s_pe, 6 * NBI)
            nc.vector.tensor_copy(out=xs_sb[:, 0:512], in_=xs_ps0)
            nc.vector.tensor_copy(
                out=xs_sb[:, 512:1024], in_=xs_ps1
            ).then_inc(s_dve, 1)

    return nc


_CACHE = {}


def _get_program():
    if "nc" not in _CACHE:
        _CACHE["nc"] = build_program()
    return _CACHE["nc"]


def _run(inputs, trace=False):
    x = np.ascontiguousarray(np.asarray(inputs["x"], dtype=np.float32))
    idxs = np.asarray(inputs["idxs"])
    W1 = np.asarray(inputs["W1"], dtype=np.float32)
    b1 = np.asarray(inputs["b1"], dtype=np.float32)
    Wa1 = np.asarray(inputs["Wa1"], dtype=np.float32)
    ba1 = np.asarray(inputs["ba1"], dtype=np.float32)
    Wa2 = np.asarray(inputs["Wa2"], dtype=np.float32)
    Wp = np.asarray(inputs["Wp"], dtype=np.float32)
    bp = np.asarray(inputs["bp"], dtype=np.float32)

    wc = (W1 @ Wa1).astype(np.float32)                      # [L, F]
    bc = (b1 @ Wa1 + ba1).astype(np.float32).reshape(F, 1)  # [F, 1]
    wa2 = Wa2.reshape(F, 1).astype(np.float32)
    segf = idxs.astype(np.float32)
    ident = np.eye(128, dtype=np.float32)
    iota64 = np.broadcast_to(
        np.arange(S, dtype=np.float32)[None, :], (128, S)
    ).copy()
    ones = np.ones((128, 2), dtype=np.float32)

    nc = _get_program()
    in_maps = []
    for c in range(N_CORES):
        in_maps.append(
            {
                "x": x[c * R : (c + 1) * R],
                "segf": np.ascontiguousarray(
                    segf[c * R : (c + 1) * R].reshape(R // 128, 128).T
                ),
                "wc": wc,
                "bc": bc,
                "wa2": wa2,
                "ident": ident,
                "iota64": iota64,
                "ones": ones,
            }
        )
    res = run_bass_kernel_spmd(
        nc, in_maps, list(range(N_CORES)), trace=trace,
    )

    xs_tot = np.zeros((S, L), dtype=np.float64)
    es_tot = np.zeros((S,), dtype=np.float64)
    for r in res.results:
        xs_tot += r["xs"].astype(np.float64)
        es_tot += r["es"].reshape(-1).astype(np.float64)

    # host epilogue (tiny [64, .] math)
    Mw = xs_tot / es_tot[:, None]
    M = Mw @ W1.astype(np.float64) + b1.astype(np.float64)
    proj = M @ Wp.astype(np.float64) + bp.astype(np.float64)
    nrm = np.linalg.norm(proj, axis=1, keepdims=True)
    proj = proj / np.maximum(nrm, 1e-12)
    return (M.astype(np.float32), proj.astype(np.float32)), res


def kernel(**inputs):
    out, _ = _run(inputs, trace=bool(int(os.environ.get("KERNEL_TRACE", "0"))))
    return out
